# revision 6
# baseline (speedup 1.0000x reference)
"""CompoundHeadAttention TRN2 kernel.

Full-input contract: kernel(**inputs) takes the unsharded tensors from
setup_inputs() and returns the full [1, 2048, 2048] float32 output.

Sharding (8 cores, tensor-parallel over the HK=8 kv heads):
  core h owns kv head h: its Wq/Wk/Wv column slice, its WG[h]/bG[h], and
  Wfc row-slice [h*256:(h+1)*256, :].  Each core computes its head's
  attention + its partial FC output [2048, 2048]; the host sums the 8
  partials and adds bfc (the "all-reduce" of the row-sharded FC).

Device-side math per core (N=2048, E=2048, D=64, G=4):
  QT  [128, n] = dup([Wq_h|Wq_h]^T q^T) + bq        (fp16 matmul, fp32 psum)
  KT  [128, s] = dup                                 (dup rows = row-tiling feed)
  V   [s, 64]  via VT matmul + PE transpose, ones column appended (M=65)
  QgT [128, n] per g-pair via WG row-tiled matmuls
  ST  [s=128, n=512] = KT_chunk^T QgT  (two row-tiled K=64 matmuls)
  PT  = exp(8*ST)  (ACT, scale folds the D**-0.5 softmax scale)
  causal mask: gpsimd affine_select zeroes PT where n < s (diagonal chunks)
  PV  [65, n] += Vones_chunk^T PT  (row 64 = softmax denominators)
  hidden = PV[0:64] * recip(PV[64])  (DVE + gpsimd partition_broadcast)
  out_partial[n, :] = hidden01^T Wfc[0:128] + hidden23^T Wfc[128:256]

Matmul dtypes: fp16 for the projections (inputs shipped as fp16),
float32r (1 cycle/row at N=512) for everything downstream.
"""

import os
import sys

import numpy as np

if "/opt/trn_rl_repo" not in sys.path and os.path.isdir("/opt/trn_rl_repo"):
    sys.path.insert(0, "/opt/trn_rl_repo")

import concourse.bass as bass  # noqa: E402
import concourse.mybir as mybir  # noqa: E402
import concourse.tile as tile  # noqa: E402
from concourse import bacc  # noqa: E402
from concourse import bass_utils  # noqa: E402

F32 = mybir.dt.float32
F32R = mybir.dt.float32r
F16 = mybir.dt.float16
AF = mybir.ActivationFunctionType

N = 2048
E = 2048
HK = 8
D = 64
G = 4
NB = 4        # 512-wide n-windows
SC_PER_NB = 4  # 128-wide s-chunks per window
NEG = -1e30


def build_program():
    nc = bacc.Bacc("TRN2", target_bir_lowering=False, debug=False,
                   enable_asserts=False)

    # ---- DRAM I/O ----
    qT = nc.dram_tensor("qT", [E, N], F16, kind="ExternalInput").ap()
    kT = nc.dram_tensor("kT", [E, N], F16, kind="ExternalInput").ap()
    vT = nc.dram_tensor("vT", [E, N], F16, kind="ExternalInput").ap()
    # weight chunk layout: [128, 16*M] — e-chunk ec occupies cols [M*ec, M*ec+M)
    wq = nc.dram_tensor("wq", [128, 16 * 128], F16, kind="ExternalInput").ap()
    wk = nc.dram_tensor("wk", [128, 16 * 128], F16, kind="ExternalInput").ap()
    wv = nc.dram_tensor("wv", [128, 16 * 64], F16, kind="ExternalInput").ap()
    bq2 = nc.dram_tensor("bq2", [128, 1], F32, kind="ExternalInput").ap()
    bk2 = nc.dram_tensor("bk2", [128, 1], F32, kind="ExternalInput").ap()
    bvv = nc.dram_tensor("bvv", [64, 1], F32, kind="ExternalInput").ap()
    wg = nc.dram_tensor("wg", [128, 256], F32R, kind="ExternalInput").ap()
    bg01 = nc.dram_tensor("bg01", [128, 1], F32, kind="ExternalInput").ap()
    bg23 = nc.dram_tensor("bg23", [128, 1], F32, kind="ExternalInput").ap()
    wfc = nc.dram_tensor("wfc", [256, E], F32R, kind="ExternalInput").ap()
    ident = nc.dram_tensor("ident", [128, 128], F32, kind="ExternalInput").ap()
    out = nc.dram_tensor("out", [N, E], F32, kind="ExternalOutput").ap()

    with tile.TileContext(nc) as tc:
        build_tile_kernel(tc, qT=qT, kT=kT, vT=vT, wq=wq, wk=wk, wv=wv,
                          bq2=bq2, bk2=bk2, bvv=bvv, wg=wg, bg01=bg01,
                          bg23=bg23, wfc=wfc, ident=ident, out=out)
    nc.compile()
    return nc


def build_tile_kernel(tc, *, qT, kT, vT, wq, wk, wv, bq2, bk2, bvv, wg,
                      bg01, bg23, wfc, ident, out):
    nc = tc.nc

    import contextlib
    ctx = contextlib.ExitStack()
    ctx.__enter__()
    cp = ctx.enter_context(tc.tile_pool(name="persist", bufs=1))

    def ptile(shape, dtype, name):
        return cp.tile(shape, dtype, tag=name, name=name)

    # ---- persistent constants in SBUF ----
    wq_sb = ptile([128, 16 * 128], F16, "wq_sb")
    wk_sb = ptile([128, 16 * 128], F16, "wk_sb")
    wv_sb = ptile([128, 16 * 64], F16, "wv_sb")
    wg_sb = ptile([128, 256], F32R, "wg_sb")
    wfc0_sb = ptile([128, E], F32R, "wfc0_sb")
    wfc1_sb = ptile([128, E], F32R, "wfc1_sb")
    id_sb = ptile([128, 128], F32, "id_sb")
    bq_sb = ptile([128, 1], F32, "bq_sb")
    bk_sb = ptile([128, 1], F32, "bk_sb")
    bv_sb = ptile([64, 1], F32, "bv_sb")
    bg01_sb = ptile([128, 1], F32, "bg01_sb")
    bg23_sb = ptile([128, 1], F32, "bg23_sb")
    ones_sb = ptile([128, 1], F32, "ones_sb")
    nc.vector.memset(ones_sb[:], 1.0)

    nc.sync.dma_start(wq_sb[:], wq[:])
    nc.sync.dma_start(wk_sb[:], wk[:])
    nc.sync.dma_start(wv_sb[:], wv[:])
    nc.sync.dma_start(wg_sb[:], wg[:])
    nc.sync.dma_start(wfc0_sb[:], wfc[0:128, :])
    nc.sync.dma_start(wfc1_sb[:], wfc[128:256, :])
    nc.sync.dma_start(id_sb[:], ident[:])
    nc.sync.dma_start(bq_sb[:], bq2[:])
    nc.sync.dma_start(bk_sb[:], bk2[:])
    nc.sync.dma_start(bv_sb[:], bvv[:])
    nc.sync.dma_start(bg01_sb[:], bg01[:])
    nc.sync.dma_start(bg23_sb[:], bg23[:])

    # per-window persistent activations
    qt_w = [ptile([128, 512], F32R, f"qt{j}") for j in range(NB)]
    kt_w = [ptile([128, 512], F32R, f"kt{j}") for j in range(NB)]
    vo_w = [ptile([128, 4 * 65], F32R, f"vo{j}") for j in range(NB)]
    qg01_w = [ptile([128, 512], F32R, f"qg01_{j}") for j in range(NB)]
    qg23_w = [ptile([128, 512], F32R, f"qg23_{j}") for j in range(NB)]
    hid01_w = [ptile([128, 512], F32R, f"hid01_{j}") for j in range(NB)]
    hid23_w = [ptile([128, 512], F32R, f"hid23_{j}") for j in range(NB)]

    with ctx:
        in_pool = ctx.enter_context(tc.tile_pool(name="in_pool", bufs=6))
        vt_pool = ctx.enter_context(tc.tile_pool(name="vt_pool", bufs=2))
        pt_pool = ctx.enter_context(tc.tile_pool(name="pt_pool", bufs=3))
        rec_pool = ctx.enter_context(tc.tile_pool(name="rec_pool", bufs=2))
        fco_pool = ctx.enter_context(tc.tile_pool(name="fco_pool", bufs=4))
        misc_ps = ctx.enter_context(
            tc.tile_pool(name="misc_ps", bufs=2, space="PSUM"))
        st_ps = ctx.enter_context(
            tc.tile_pool(name="st_ps", bufs=2, space="PSUM"))
        pv_ps = ctx.enter_context(
            tc.tile_pool(name="pv_ps", bufs=2, space="PSUM"))

        for j in range(NB):
            ncol = bass.ts(j, 512)  # this window's n/s columns

            # ---- Q projection (window j), output dup'd across partitions ----
            q_ps = misc_ps.tile([128, 512], F32, tag="mm")
            for ec in range(16):
                q_in = in_pool.tile([128, 512], F16, tag="qin")
                nc.sync.dma_start(q_in[:], qT[bass.ts(ec, 128), ncol])
                nc.tensor.matmul(q_ps[:], wq_sb[:, bass.ts(ec, 128)], q_in[:],
                                 start=(ec == 0), stop=(ec == 15))
            nc.vector.tensor_scalar_add(qt_w[j][:], q_ps[:], bq_sb[:])

            # ---- K projection (window j) ----
            k_ps = misc_ps.tile([128, 512], F32, tag="mm")
            for ec in range(16):
                k_in = in_pool.tile([128, 512], F16, tag="kin")
                nc.sync.dma_start(k_in[:], kT[bass.ts(ec, 128), ncol])
                nc.tensor.matmul(k_ps[:], wk_sb[:, bass.ts(ec, 128)], k_in[:],
                                 start=(ec == 0), stop=(ec == 15))
            nc.vector.tensor_scalar_add(kt_w[j][:], k_ps[:], bk_sb[:])

            # ---- V projection (window j): VT then PE-transpose to V ----
            v_ps = misc_ps.tile([64, 512], F32, tag="mm")
            for ec in range(16):
                v_in = in_pool.tile([128, 512], F16, tag="vin")
                nc.sync.dma_start(v_in[:], vT[bass.ts(ec, 128), ncol])
                nc.tensor.matmul(v_ps[:], wv_sb[:, bass.ts(ec, 64)], v_in[:],
                                 start=(ec == 0), stop=(ec == 15))
            vt_sb = vt_pool.tile([64, 512], F32, tag="vt")
            nc.vector.tensor_scalar_add(vt_sb[:], v_ps[:], bv_sb[:])
            tr_ps = misc_ps.tile([128, 256], F32, tag="mm")
            for t in range(4):
                nc.tensor.transpose(tr_ps[:, bass.ts(t, 64)],
                                    vt_sb[:, bass.ts(t, 128)],
                                    id_sb[0:64, 0:64])
            for t in range(4):
                nc.vector.tensor_copy(vo_w[j][:, t * 65:t * 65 + 64],
                                      tr_ps[:, bass.ts(t, 64)])
                nc.vector.tensor_copy(vo_w[j][:, t * 65 + 64:t * 65 + 65],
                                      ones_sb[:])

            # ---- G transform (window j), row-tiled pair01 / pair23 ----
            g01_ps = misc_ps.tile([128, 512], F32, tag="mm")
            g23_ps = misc_ps.tile([128, 512], F32, tag="mm")
            nc.tensor.matmul(g01_ps[:], wg_sb[0:64, 0:128],
                             qt_w[j][0:64, :], start=True, stop=True)
            nc.tensor.matmul(g23_ps[:], wg_sb[64:128, 128:256],
                             qt_w[j][64:128, :], start=True, stop=True)
            nc.vector.tensor_scalar_add(qg01_w[j][:], g01_ps[:], bg01_sb[:])
            nc.vector.tensor_scalar_add(qg23_w[j][:], g23_ps[:], bg23_sb[:])

            # ---- attention for window j ----
            klast = 4 * j + 3
            for (qg, hid) in ((qg01_w[j], hid01_w[j]),
                              (qg23_w[j], hid23_w[j])):
                pv_a = pv_ps.tile([65, 512], F32, tag="pv")
                pv_b = pv_ps.tile([65, 512], F32, tag="pv")
                for k in range(klast + 1):
                    kt_c = kt_w[k // 4]
                    ks = bass.ts(k % 4, 128)
                    st = st_ps.tile([128, 1024], F32, tag="st")
                    nc.tensor.matmul(st[:, 0:512], kt_c[0:64, ks],
                                     qg[0:64, :], start=True, stop=True)
                    nc.tensor.matmul(st[:, 512:1024], kt_c[64:128, ks],
                                     qg[64:128, :], start=True, stop=True)
                    pt = pt_pool.tile([128, 1024], F32R, tag="pt")
                    nc.scalar.activation(pt[:], st[:], AF.Exp, scale=8.0)
                    if k >= 4 * j:
                        # zero masked probabilities: keep where c >= s + 128*i
                        i = k - 4 * j
                        pt3 = pt[:].rearrange("p (h c) -> p h c", c=512)
                        nc.gpsimd.affine_select(
                            out=pt3, in_=pt3,
                            compare_op=mybir.AluOpType.is_ge,
                            fill=0.0, base=-128 * i,
                            pattern=[[0, 2], [1, 512]],
                            channel_multiplier=-1)
                    vo_c = vo_w[k // 4]
                    vsl = vo_c[:, (k % 4) * 65:(k % 4) * 65 + 65]
                    nc.tensor.matmul(pv_a[:], vsl, pt[:, 0:512],
                                     start=(k == 0), stop=(k == klast))
                    nc.tensor.matmul(pv_b[:], vsl, pt[:, 512:1024],
                                     start=(k == 0), stop=(k == klast))
                # normalize: hidden[g-half] = pv[0:64] * 1/pv[64]
                for half, pv in ((0, pv_a), (1, pv_b)):
                    rec = rec_pool.tile([1, 512], F32, tag="rec")
                    nc.vector.reciprocal(rec[:], pv[64:65, :])
                    recr = rec_pool.tile([64, 512], F32, tag="recr")
                    nc.gpsimd.partition_broadcast(recr[:], rec[:])
                    nc.vector.tensor_mul(hid[half * 64:half * 64 + 64, :],
                                         pv[0:64, :], recr[:])

            # ---- FC partial for window j's rows ----
            for m in range(4):
                msl = bass.ts(m, 128)
                for eo in range(4):
                    fc_ps = misc_ps.tile([128, 512], F32, tag="mm")
                    nc.tensor.matmul(fc_ps[:], hid01_w[j][:, msl],
                                     wfc0_sb[:, bass.ts(eo, 512)],
                                     start=True, stop=False)
                    nc.tensor.matmul(fc_ps[:], hid23_w[j][:, msl],
                                     wfc1_sb[:, bass.ts(eo, 512)],
                                     start=False, stop=True)
                    ot = fco_pool.tile([128, 512], F32, tag="fco")
                    nc.vector.tensor_copy(ot[:], fc_ps[:])
                    nc.sync.dma_start(out[512 * j + 128 * m:
                                          512 * j + 128 * m + 128,
                                          bass.ts(eo, 512)], ot[:])


def shard_inputs(inputs):
    """full inputs -> list of 8 per-core in_maps (numpy, device layouts)"""
    f16 = np.float16
    f32 = np.float32
    q = np.asarray(inputs["q"], f32)[0]
    k = np.asarray(inputs["k"], f32)[0]
    v = np.asarray(inputs["v"], f32)[0]
    Wq = np.asarray(inputs["Wq"], f32)
    Wk = np.asarray(inputs["Wk"], f32)
    Wv = np.asarray(inputs["Wv"], f32)
    bq = np.asarray(inputs["bq"], f32)
    bk = np.asarray(inputs["bk"], f32)
    bv = np.asarray(inputs["bv"], f32)
    WG = np.asarray(inputs["WG"], f32)
    bG = np.asarray(inputs["bG"], f32)
    Wfc = np.asarray(inputs["Wfc"], f32)

    qT = np.ascontiguousarray(q.T.astype(f16))
    kT = np.ascontiguousarray(k.T.astype(f16))
    vT = np.ascontiguousarray(v.T.astype(f16))
    ident = np.eye(128, dtype=f32)

    def chunked(w):
        # [E, M] -> [128, 16*M]: e-chunk ec at cols [M*ec, M*ec+M)
        M = w.shape[1]
        return np.ascontiguousarray(
            w.reshape(16, 128, M).transpose(1, 0, 2).reshape(128, 16 * M))

    maps = []
    for h in range(HK):
        sl = slice(h * D, (h + 1) * D)
        wq_h = Wq[:, sl]
        wk_h = Wk[:, sl]
        wv_h = Wv[:, sl]
        m = {
            "qT": qT, "kT": kT, "vT": vT,
            "wq": chunked(np.concatenate([wq_h, wq_h], 1)).astype(f16),
            "wk": chunked(np.concatenate([wk_h, wk_h], 1)).astype(f16),
            "wv": chunked(wv_h).astype(f16),
            "bq2": np.concatenate([bq[sl], bq[sl]]).reshape(128, 1).copy(),
            "bk2": np.concatenate([bk[sl], bk[sl]]).reshape(128, 1).copy(),
            "bvv": bv[sl].reshape(64, 1).copy(),
            "wg": np.concatenate([WG[h], WG[h]], 0).copy(),  # [128, 256]
            "bg01": bG[h, 0:128].reshape(128, 1).copy(),
            "bg23": bG[h, 128:256].reshape(128, 1).copy(),
            "wfc": Wfc[h * 256:(h + 1) * 256, :].copy(),
            "ident": ident,
        }
        maps.append(m)
    return maps


_compiled = None
last_results = None


def get_compiled():
    global _compiled
    if _compiled is None:
        _compiled = build_program()
    return _compiled


def kernel(**inputs):
    global last_results
    nc = get_compiled()
    in_maps = shard_inputs(inputs)
    last_results = bass_utils.run_bass_kernel_spmd(
        nc, in_maps, core_ids=list(range(8)))
    bfc = np.asarray(inputs["bfc"], np.float32)
    acc = np.zeros((N, E), np.float64)
    for res in last_results.results:
        acc += res["out"].astype(np.float64)
    full = (acc + bfc[None, :].astype(np.float64)).astype(np.float32)
    return full.reshape(1, N, E)


# revision 8
# speedup vs baseline: 1.8801x; 1.8801x over previous
"""CompoundHeadAttention TRN2 kernel.

Full-input contract: kernel(**inputs) takes the unsharded tensors from
setup_inputs() and returns the full [1, 2048, 2048] float32 output.

Sharding (8 cores, tensor-parallel over the HK=8 kv heads):
  core h owns kv head h: its Wq/Wk/Wv column slice, its WG[h]/bG[h], and
  Wfc row-slice [h*256:(h+1)*256, :].  Each core computes its head's
  attention + its partial FC output [2048, 2048]; the host sums the 8
  partials and adds bfc (the "all-reduce" of the row-sharded FC).

Device-side math per core (N=2048, E=2048, D=64, G=4):
  QT  [128, n] = dup([Wq_h|Wq_h]^T q^T) + bq        (fp16 matmul, fp32 psum)
  KT  [128, s] = dup                                 (dup rows = row-tiling feed)
  V   [s, 64]  via VT matmul + PE transpose, ones column appended (M=65)
  QgT [128, n] per g-pair via WG row-tiled matmuls
  ST  [s=128, n=512] = KT_chunk^T QgT  (two row-tiled K=64 matmuls)
  PT  = exp(8*ST)  (ACT, scale folds the D**-0.5 softmax scale)
  causal mask: gpsimd affine_select zeroes PT where n < s (diagonal chunks)
  PV  [65, n] += Vones_chunk^T PT  (row 64 = softmax denominators)
  hidden = PV[0:64] * recip(PV[64])  (DVE + gpsimd partition_broadcast)
  out_partial[n, :] = hidden01^T Wfc[0:128] + hidden23^T Wfc[128:256]

Matmul dtypes: fp16 for the projections (inputs shipped as fp16),
float32r (1 cycle/row at N=512) for everything downstream.
"""

import os
import sys

import numpy as np

if "/opt/trn_rl_repo" not in sys.path and os.path.isdir("/opt/trn_rl_repo"):
    sys.path.insert(0, "/opt/trn_rl_repo")

import concourse.bass as bass  # noqa: E402
import concourse.mybir as mybir  # noqa: E402
import concourse.tile as tile  # noqa: E402
from concourse import bacc  # noqa: E402
from concourse import bass_utils  # noqa: E402

F32 = mybir.dt.float32
F32R = mybir.dt.float32r
F16 = mybir.dt.float16
AF = mybir.ActivationFunctionType

N = 2048
E = 2048
HK = 8
D = 64
G = 4
NB = 4        # 512-wide n-windows
SC_PER_NB = 4  # 128-wide s-chunks per window
NEG = -1e30


def build_program():
    nc = bacc.Bacc("TRN2", target_bir_lowering=False, debug=False,
                   enable_asserts=False)

    # ---- DRAM I/O ----
    qT = nc.dram_tensor("qT", [E, N], F16, kind="ExternalInput").ap()
    kT = nc.dram_tensor("kT", [E, N], F16, kind="ExternalInput").ap()
    vT = nc.dram_tensor("vT", [E, N], F16, kind="ExternalInput").ap()
    # weight chunk layout: [128, 16*M] — e-chunk ec occupies cols [M*ec, M*ec+M)
    wq = nc.dram_tensor("wq", [128, 16 * 128], F16, kind="ExternalInput").ap()
    wk = nc.dram_tensor("wk", [128, 16 * 128], F16, kind="ExternalInput").ap()
    wv = nc.dram_tensor("wv", [128, 16 * 64], F16, kind="ExternalInput").ap()
    bq2 = nc.dram_tensor("bq2", [128, 1], F32, kind="ExternalInput").ap()
    bk2 = nc.dram_tensor("bk2", [128, 1], F32, kind="ExternalInput").ap()
    bvv = nc.dram_tensor("bvv", [64, 1], F32, kind="ExternalInput").ap()
    wg = nc.dram_tensor("wg", [128, 256], F32R, kind="ExternalInput").ap()
    bg01 = nc.dram_tensor("bg01", [128, 1], F32, kind="ExternalInput").ap()
    bg23 = nc.dram_tensor("bg23", [128, 1], F32, kind="ExternalInput").ap()
    wfc = nc.dram_tensor("wfc", [256, E], F32R, kind="ExternalInput").ap()
    ident = nc.dram_tensor("ident", [128, 128], F32, kind="ExternalInput").ap()
    out = nc.dram_tensor("out", [N, E], F32, kind="ExternalOutput").ap()

    with tile.TileContext(nc) as tc:
        build_tile_kernel(tc, qT=qT, kT=kT, vT=vT, wq=wq, wk=wk, wv=wv,
                          bq2=bq2, bk2=bk2, bvv=bvv, wg=wg, bg01=bg01,
                          bg23=bg23, wfc=wfc, ident=ident, out=out)
    nc.compile()
    return nc


def build_tile_kernel(tc, *, qT, kT, vT, wq, wk, wv, bq2, bk2, bvv, wg,
                      bg01, bg23, wfc, ident, out):
    nc = tc.nc

    import contextlib
    ctx = contextlib.ExitStack()
    ctx.__enter__()
    cp = ctx.enter_context(tc.tile_pool(name="persist", bufs=1))

    def ptile(shape, dtype, name):
        return cp.tile(shape, dtype, tag=name, name=name)

    # ---- persistent constants in SBUF ----
    wq_sb = ptile([128, 16 * 128], F16, "wq_sb")
    wk_sb = ptile([128, 16 * 128], F16, "wk_sb")
    wv_sb = ptile([128, 16 * 64], F16, "wv_sb")
    wg_sb = ptile([128, 256], F32R, "wg_sb")
    wfc0_sb = ptile([128, E], F32R, "wfc0_sb")
    wfc1_sb = ptile([128, E], F32R, "wfc1_sb")
    id_sb = ptile([128, 128], F32, "id_sb")
    bq_sb = ptile([128, 1], F32, "bq_sb")
    bk_sb = ptile([128, 1], F32, "bk_sb")
    bv_sb = ptile([64, 1], F32, "bv_sb")
    bg01_sb = ptile([128, 1], F32, "bg01_sb")
    bg23_sb = ptile([128, 1], F32, "bg23_sb")
    ones_sb = ptile([128, 1], F32, "ones_sb")
    nc.vector.memset(ones_sb[:], 1.0)

    nc.sync.dma_start(wq_sb[:], wq[:])
    nc.sync.dma_start(wk_sb[:], wk[:])
    nc.sync.dma_start(wv_sb[:], wv[:])
    nc.sync.dma_start(wg_sb[:], wg[:])
    nc.sync.dma_start(wfc0_sb[:], wfc[0:128, :])
    nc.sync.dma_start(wfc1_sb[:], wfc[128:256, :])
    nc.sync.dma_start(id_sb[:], ident[:])
    nc.sync.dma_start(bq_sb[:], bq2[:])
    nc.sync.dma_start(bk_sb[:], bk2[:])
    nc.sync.dma_start(bv_sb[:], bvv[:])
    nc.sync.dma_start(bg01_sb[:], bg01[:])
    nc.sync.dma_start(bg23_sb[:], bg23[:])

    # per-window persistent activations
    qt_w = [ptile([128, 512], F32R, f"qt{j}") for j in range(NB)]
    kt_w = [ptile([128, 512], F32R, f"kt{j}") for j in range(NB)]
    vo_w = [ptile([128, 4 * 65], F32R, f"vo{j}") for j in range(NB)]
    qg01_w = [ptile([128, 512], F32R, f"qg01_{j}") for j in range(NB)]
    qg23_w = [ptile([128, 512], F32R, f"qg23_{j}") for j in range(NB)]
    hid01_w = [ptile([128, 512], F32R, f"hid01_{j}") for j in range(NB)]
    hid23_w = [ptile([128, 512], F32R, f"hid23_{j}") for j in range(NB)]

    with ctx:
        in_pool = ctx.enter_context(tc.tile_pool(name="in_pool", bufs=9))
        vt_pool = ctx.enter_context(tc.tile_pool(name="vt_pool", bufs=2))
        pt_pool = ctx.enter_context(tc.tile_pool(name="pt_pool", bufs=3))
        rec_pool = ctx.enter_context(tc.tile_pool(name="rec_pool", bufs=2))
        fco_pool = ctx.enter_context(tc.tile_pool(name="fco_pool", bufs=2))
        misc_ps = ctx.enter_context(
            tc.tile_pool(name="misc_ps", bufs=2, space="PSUM"))
        st_ps = ctx.enter_context(
            tc.tile_pool(name="st_ps", bufs=2, space="PSUM"))
        pv_ps = ctx.enter_context(
            tc.tile_pool(name="pv_ps", bufs=2, space="PSUM"))

        for P in range(2):  # window pairs {0,1}, {2,3}
            pcol = bass.ds(P * 1024, 1024)
            wins = (2 * P, 2 * P + 1)

            # ---- Q projection (both windows), dup'd output partitions ----
            q0_ps = misc_ps.tile([128, 512], F32, tag="mm")
            q1_ps = misc_ps.tile([128, 512], F32, tag="mm")
            for ec in range(16):
                q_in = in_pool.tile([128, 1024], F16, tag="qin")
                nc.sync.dma_start(q_in[:], qT[bass.ts(ec, 128), pcol])
                w = wq_sb[:, bass.ts(ec, 128)]
                nc.tensor.matmul(q0_ps[:], w, q_in[:, 0:512],
                                 start=(ec == 0), stop=(ec == 15))
                nc.tensor.matmul(q1_ps[:], w, q_in[:, 512:1024],
                                 start=(ec == 0), stop=(ec == 15))
            nc.vector.tensor_scalar_add(qt_w[wins[0]][:], q0_ps[:], bq_sb[:])
            nc.vector.tensor_scalar_add(qt_w[wins[1]][:], q1_ps[:], bq_sb[:])

            # ---- K projection (both windows) ----
            k0_ps = misc_ps.tile([128, 512], F32, tag="mm")
            k1_ps = misc_ps.tile([128, 512], F32, tag="mm")
            for ec in range(16):
                k_in = in_pool.tile([128, 1024], F16, tag="kin")
                nc.sync.dma_start(k_in[:], kT[bass.ts(ec, 128), pcol])
                w = wk_sb[:, bass.ts(ec, 128)]
                nc.tensor.matmul(k0_ps[:], w, k_in[:, 0:512],
                                 start=(ec == 0), stop=(ec == 15))
                nc.tensor.matmul(k1_ps[:], w, k_in[:, 512:1024],
                                 start=(ec == 0), stop=(ec == 15))
            nc.vector.tensor_scalar_add(kt_w[wins[0]][:], k0_ps[:], bk_sb[:])
            nc.vector.tensor_scalar_add(kt_w[wins[1]][:], k1_ps[:], bk_sb[:])

            # ---- V projection (both windows): VT then PE-transpose to V ----
            v0_ps = misc_ps.tile([64, 512], F32, tag="mm")
            v1_ps = misc_ps.tile([64, 512], F32, tag="mm")
            for ec in range(16):
                v_in = in_pool.tile([128, 1024], F16, tag="vin")
                nc.sync.dma_start(v_in[:], vT[bass.ts(ec, 128), pcol])
                w = wv_sb[:, bass.ts(ec, 64)]
                nc.tensor.matmul(v0_ps[:], w, v_in[:, 0:512],
                                 start=(ec == 0), stop=(ec == 15))
                nc.tensor.matmul(v1_ps[:], w, v_in[:, 512:1024],
                                 start=(ec == 0), stop=(ec == 15))
            for wi, v_ps in ((wins[0], v0_ps), (wins[1], v1_ps)):
                vt_sb = vt_pool.tile([64, 512], F32, tag="vt")
                nc.vector.tensor_scalar_add(vt_sb[:], v_ps[:], bv_sb[:])
                tr_ps = misc_ps.tile([128, 256], F32, tag="mm")
                for t in range(4):
                    nc.tensor.transpose(tr_ps[:, bass.ts(t, 64)],
                                        vt_sb[:, bass.ts(t, 128)],
                                        id_sb[0:64, 0:64])
                for t in range(4):
                    nc.vector.tensor_copy(vo_w[wi][:, t * 65:t * 65 + 64],
                                          tr_ps[:, bass.ts(t, 64)])
                    nc.vector.tensor_copy(vo_w[wi][:, t * 65 + 64:t * 65 + 65],
                                          ones_sb[:])

            # ---- G transform per window, row-tiled pair01 / pair23 ----
            for wi in wins:
                g01_ps = misc_ps.tile([128, 512], F32, tag="mm")
                g23_ps = misc_ps.tile([128, 512], F32, tag="mm")
                nc.tensor.matmul(g01_ps[:], wg_sb[0:64, 0:128],
                                 qt_w[wi][0:64, :], start=True, stop=True)
                nc.tensor.matmul(g23_ps[:], wg_sb[64:128, 128:256],
                                 qt_w[wi][64:128, :], start=True, stop=True)
                nc.vector.tensor_scalar_add(qg01_w[wi][:], g01_ps[:], bg01_sb[:])
                nc.vector.tensor_scalar_add(qg23_w[wi][:], g23_ps[:], bg23_sb[:])

            # ---- attention per window ----
            for wi in wins:
                j = wi
                klast = 4 * j + 3
                for (qg, hid) in ((qg01_w[j], hid01_w[j]),
                                  (qg23_w[j], hid23_w[j])):
                    pv_a = pv_ps.tile([65, 512], F32, tag="pv")
                    pv_b = pv_ps.tile([65, 512], F32, tag="pv")
                    for k in range(klast + 1):
                        kt_c = kt_w[k // 4]
                        ks = bass.ts(k % 4, 128)
                        # causal trim: diagonal chunk k covers only n-cols
                        # [off, 512) of the window (off = 128*(k-4j));
                        # leading 128 of those get the triangle mask.
                        off = max(0, 128 * (k - 4 * j))
                        W = 512 - off
                        st = st_ps.tile([128, 1024], F32, tag="st")
                        nc.tensor.matmul(st[:, off:512], kt_c[0:64, ks],
                                         qg[0:64, off:512],
                                         start=True, stop=True)
                        nc.tensor.matmul(st[:, 512 + off:1024],
                                         kt_c[64:128, ks],
                                         qg[64:128, off:512],
                                         start=True, stop=True)
                        pt = pt_pool.tile([128, 1024], F32R, tag="pt")
                        st3 = st[:].rearrange("p (h c) -> p h c", c=512)
                        pt3 = pt[:].rearrange("p (h c) -> p h c", c=512)
                        nc.scalar.activation(pt3[:, :, off:512],
                                             st3[:, :, off:512],
                                             AF.Exp, scale=8.0)
                        if k >= 4 * j:
                            # triangle mask on first 128 valid cols:
                            # keep where (off + c') >= s + off  i.e. c' >= s
                            nc.gpsimd.affine_select(
                                out=pt3[:, :, off:off + 128],
                                in_=pt3[:, :, off:off + 128],
                                compare_op=mybir.AluOpType.is_ge,
                                fill=0.0, base=0,
                                pattern=[[0, 2], [1, 128]],
                                channel_multiplier=-1)
                        vo_c = vo_w[k // 4]
                        vsl = vo_c[:, (k % 4) * 65:(k % 4) * 65 + 65]
                        nc.tensor.matmul(pv_a[:, off:512], vsl,
                                         pt[:, off:512],
                                         start=(k == 0), stop=(k == klast))
                        nc.tensor.matmul(pv_b[:, off:512], vsl,
                                         pt[:, 512 + off:1024],
                                         start=(k == 0), stop=(k == klast))
                    # normalize: hidden[g-half] = pv[0:64] * 1/pv[64]
                    for half, pv in ((0, pv_a), (1, pv_b)):
                        rec = rec_pool.tile([1, 512], F32, tag="rec")
                        nc.vector.reciprocal(rec[:], pv[64:65, :])
                        recr = rec_pool.tile([64, 512], F32, tag="recr")
                        nc.gpsimd.partition_broadcast(recr[:], rec[:])
                        nc.vector.tensor_mul(hid[half * 64:half * 64 + 64, :],
                                             pv[0:64, :], recr[:])

            # ---- FC partial rows for both windows (staged 1MB out DMAs) ----
            for wi in wins:
                j = wi
                for m in range(4):
                    msl = bass.ts(m, 128)
                    stage = fco_pool.tile([128, 2048], F32, tag="fco")
                    for eo in range(4):
                        fc_ps = misc_ps.tile([128, 512], F32, tag="mm")
                        nc.tensor.matmul(fc_ps[:], hid01_w[j][:, msl],
                                         wfc0_sb[:, bass.ts(eo, 512)],
                                         start=True, stop=False)
                        nc.tensor.matmul(fc_ps[:], hid23_w[j][:, msl],
                                         wfc1_sb[:, bass.ts(eo, 512)],
                                         start=False, stop=True)
                        nc.vector.tensor_copy(stage[:, bass.ts(eo, 512)],
                                              fc_ps[:])
                    nc.sync.dma_start(
                        out[512 * j + 128 * m: 512 * j + 128 * m + 128, :],
                        stage[:])


def shard_inputs(inputs):
    """full inputs -> list of 8 per-core in_maps (numpy, device layouts)"""
    f16 = np.float16
    f32 = np.float32
    q = np.asarray(inputs["q"], f32)[0]
    k = np.asarray(inputs["k"], f32)[0]
    v = np.asarray(inputs["v"], f32)[0]
    Wq = np.asarray(inputs["Wq"], f32)
    Wk = np.asarray(inputs["Wk"], f32)
    Wv = np.asarray(inputs["Wv"], f32)
    bq = np.asarray(inputs["bq"], f32)
    bk = np.asarray(inputs["bk"], f32)
    bv = np.asarray(inputs["bv"], f32)
    WG = np.asarray(inputs["WG"], f32)
    bG = np.asarray(inputs["bG"], f32)
    Wfc = np.asarray(inputs["Wfc"], f32)

    qT = np.ascontiguousarray(q.T.astype(f16))
    kT = np.ascontiguousarray(k.T.astype(f16))
    vT = np.ascontiguousarray(v.T.astype(f16))
    ident = np.eye(128, dtype=f32)

    def chunked(w):
        # [E, M] -> [128, 16*M]: e-chunk ec at cols [M*ec, M*ec+M)
        M = w.shape[1]
        return np.ascontiguousarray(
            w.reshape(16, 128, M).transpose(1, 0, 2).reshape(128, 16 * M))

    maps = []
    for h in range(HK):
        sl = slice(h * D, (h + 1) * D)
        wq_h = Wq[:, sl]
        wk_h = Wk[:, sl]
        wv_h = Wv[:, sl]
        m = {
            "qT": qT, "kT": kT, "vT": vT,
            "wq": chunked(np.concatenate([wq_h, wq_h], 1)).astype(f16),
            "wk": chunked(np.concatenate([wk_h, wk_h], 1)).astype(f16),
            "wv": chunked(wv_h).astype(f16),
            "bq2": np.concatenate([bq[sl], bq[sl]]).reshape(128, 1).copy(),
            "bk2": np.concatenate([bk[sl], bk[sl]]).reshape(128, 1).copy(),
            "bvv": bv[sl].reshape(64, 1).copy(),
            "wg": np.concatenate([WG[h], WG[h]], 0).copy(),  # [128, 256]
            "bg01": bG[h, 0:128].reshape(128, 1).copy(),
            "bg23": bG[h, 128:256].reshape(128, 1).copy(),
            "wfc": Wfc[h * 256:(h + 1) * 256, :].copy(),
            "ident": ident,
        }
        maps.append(m)
    return maps


_compiled = None
last_results = None


def get_compiled():
    global _compiled
    if _compiled is None:
        _compiled = build_program()
    return _compiled


def kernel(**inputs):
    global last_results
    nc = get_compiled()
    in_maps = shard_inputs(inputs)
    last_results = bass_utils.run_bass_kernel_spmd(
        nc, in_maps, core_ids=list(range(8)))
    bfc = np.asarray(inputs["bfc"], np.float32)
    acc = np.zeros((N, E), np.float64)
    for res in last_results.results:
        acc += res["out"].astype(np.float64)
    full = (acc + bfc[None, :].astype(np.float64)).astype(np.float32)
    return full.reshape(1, N, E)
